# revision 1
# baseline (speedup 1.0000x reference)
"""Trainium2 Bass kernel for a DiT-style decoder block (adaLN modulation +
attention with QK-RMSNorm + MLP).  B=2, L=2048, D=768, H=12, FF=3072, fp32.

Sharding: 8 cores = 2 batches x 4 sequence-quarters.  Each core computes
adaLN(s_att/sh_att) + LayerNorm + K/V for its batch's full 2048 tokens
(4x replicated work, no collectives) and attention / to_out / MLP for its
own 512 query tokens only.

Device layout is feature-major ([channel on partitions, tokens on free]):
the host pre-transposes x/c and post-transposes outputs, so the device
needs no transposes.  Matmuls run as float32r (full PE rate).  Per-token
reductions (LayerNorm / RMSNorm / softmax denominators) are done with
ones-vector matmuls on the PE; per-token scalars are broadcast across
partitions with K=1 matmuls.  Softmax denominators come free from the
attention A@V matmul via a 65-column-per-head V layout ([v_h | 1]);
softmax skips max-subtraction (scores are O(8), exp is safe in fp32).
The V projection bias is folded into an effective to_out bias on the host
(softmax rows sum to 1, so it commutes through attention).
"""

import sys

if '/opt/trn_rl_repo' not in sys.path:
    sys.path.insert(0, '/opt/trn_rl_repo')

import contextlib
import os

import numpy as np

import concourse.bass as bass
import concourse.mybir as mybir
import concourse.tile as tile
from concourse import bacc
from concourse.bass_utils import run_bass_kernel_spmd

F32 = mybir.dt.float32
F32R = mybir.dt.float32r
AF = mybir.ActivationFunctionType
OP = mybir.AluOpType

B, L, D, HEADS, DH, FF = 2, 2048, 768, 12, 64, 3072
P = 128
DC = D // P              # 6
FFC = FF // P            # 24
KC = L // P              # 16 key chunks
N_CORES = 8
QT = L // 4              # 512
NG = L // 512            # 4 token groups
SQRT_D = float(np.sqrt(D))
SCALE = DH ** -0.5
LN_EPS = 1e-6
STAGE = int(os.environ.get('BASS_STAGE_LIMIT', '6'))
SIM_COMPAT = bool(os.environ.get("BASS_SIM_COMPAT"))
D_SUB = int(os.environ.get('BASS_D_SUB', '3'))

PACKS = ("one_p_bada_s", "bada_sh", "bada_ga", "one_p_bada_sm", "bada_shm",
         "bada_gm", "bk", "bq", "bo_eff", "go_s", "gq_s", "gk_s", "b2")


def r32(ap):
    return ap.bitcast(F32R)


def _emit(nc):
    din = {}

    def dram_in(name, shape):
        din[name] = nc.dram_tensor(name, shape, F32, kind="ExternalInput").ap()
        return din[name]

    xT = dram_in("xT", [D, L])
    cT = dram_in("cT", [D, L])
    xT_own = dram_in("xT_own", [D, QT])       # clean f32 copy for the residual
    Wada = dram_in("Wada", [D, 6 * D])
    Wqkv = dram_in("Wqkv", [D, 3 * D])
    Wo = dram_in("Wo", [D, D])
    W1 = dram_in("W1", [D, FF])
    W2 = dram_in("W2", [FF, D])
    ONES = dram_in("ONES", [P, 512])
    for name in PACKS:
        dram_in(name, [P, FFC if name == "b1" else DC])
    dram_in("b1", [P, FFC])

    outT = nc.dram_tensor("outT", [D, QT], F32, kind="ExternalOutput").ap()

    xT_c = xT.rearrange("(j p) t -> p j t", p=P)
    cT_c = cT.rearrange("(j p) t -> p j t", p=P)
    xTo_c = xT_own.rearrange("(j p) t -> p j t", p=P)
    Wada_c = Wada.rearrange("(k p) n -> p k n", p=P)
    Wqkv_c = Wqkv.rearrange("(k p) n -> p k n", p=P)
    Wo_c = Wo.rearrange("(k p) n -> p k n", p=P)
    W1_c = W1.rearrange("(k p) n -> p k n", p=P)
    W2_c = W2.rearrange("(f p) n -> p f n", p=P)
    outT_c = outT.rearrange("(j p) t -> p j t", p=P)

    with tile.TileContext(nc) as tc, contextlib.ExitStack() as ctx:
        const = ctx.enter_context(tc.tile_pool(name="const", bufs=1))
        stat = ctx.enter_context(tc.tile_pool(name="stat", bufs=1))
        rows = ctx.enter_context(tc.tile_pool(name="rows", bufs=2))
        dramp = ctx.enter_context(tc.tile_pool(name="dram", bufs=1, space="DRAM"))

        ones = const.tile([P, 512], F32, tag="ones")
        nc.sync.dma_start(r32(ones[:]), r32(ONES))
        onesr = ones[0:1, :]
        onesc = ones[:, 0:1]

        pk = {}
        for name in PACKS:
            t = const.tile([P, DC], F32, tag=name)
            nc.sync.dma_start(t[:], din[name])
            pk[name] = t
        b1_pp = const.tile([P, FFC], F32, tag="b1")
        nc.sync.dma_start(b1_pp[:], din["b1"])

        sc_own = const.tile([P, DC, QT], F32, tag="sc_own")
        htmp = dramp.tile([P, DC, L], F32, tag="htmp")

        def stat_sum(dst_ps, src_f32r, start, stop):
            nc.tensor.matmul(dst_ps, r32(onesc), src_f32r, start=start, stop=stop)

        def bcast_row(dst_ps, row_f32r, m=P):
            nc.tensor.matmul(dst_ps, r32(ones[0:1, 0:m]), row_f32r,
                             start=True, stop=True)

        def publish(row_f32):
            t = rows.tile([1, 512], F32, tag="pub")
            nc.vector.tensor_copy(r32(t[:]), row_f32[:])
            return t

        def recip_rows(src_ps, clamp, scale_to_sqrt=True):
            """rows = 1/max(sqrt(ps), clamp) (or 1/ps), published to f32r."""
            a_row = stat.tile([1, 512], F32, tag="srowA")
            b_row = stat.tile([1, 512], F32, tag="srowB")
            if scale_to_sqrt:
                nc.scalar.activation(a_row[:], src_ps, AF.Sqrt)
            else:
                nc.scalar.activation(a_row[:], src_ps, AF.Copy)
            if clamp is not None:
                nc.vector.tensor_scalar_max(a_row[:], a_row[:], clamp)
            nc.vector.reciprocal(b_row[:], a_row[:])
            return publish(b_row)

        # =========== Stage A: silu(c), LN1, mod1, h -> DRAM ===========
        with tc.tile_pool(name="pA", bufs=2) as pA, \
             tc.tile_pool(name="pA1", bufs=1) as pA1, \
             tc.tile_pool(name="psA", bufs=2, space="PSUM") as psA, \
             tc.tile_pool(name="psAs", bufs=1, space="PSUM") as psAs, \
             tc.tile_pool(name="psAb", bufs=1, space="PSUM") as psAb:
            wada1 = pA1.tile([P, DC, 2 * D], F32, tag="wada1")
            nc.sync.dma_start(r32(wada1[:]), r32(Wada_c[:, :, 0:2 * D]))
            for g in range(NG):
                gs = slice(g * 512, (g + 1) * 512)
                c_g = pA.tile([P, DC, 512], F32, tag="c_g")
                x_g = pA.tile([P, DC, 512], F32, tag="x_g")
                nc.sync.dma_start(r32(c_g[:]), r32(cT_c[:, :, gs]))
                nc.sync.dma_start(r32(x_g[:]), r32(xT_c[:, :, gs]))
                sc_g = pA1.tile([P, DC, 512], F32, tag="sc_g")
                if SIM_COMPAT:
                    nc.scalar.activation(r32(sc_g[:]), c_g[:], AF.Sigmoid)
                    nc.vector.tensor_tensor(r32(sc_g[:]), sc_g[:], c_g[:], OP.mult)
                else:
                    nc.scalar.activation(r32(sc_g[:]), c_g[:], AF.Silu)
                if g == 0:
                    nc.vector.tensor_copy(r32(sc_own[:]), sc_g[:])
                ps_s = psAs.tile([1, 512], F32, tag="ps_s")
                ps_ss = psAs.tile([1, 512], F32, tag="ps_ss")
                for j in range(DC):
                    stat_sum(ps_s[:], r32(x_g[:, j, :]), j == 0, j == DC - 1)
                for j in range(DC):
                    xsq = pA.tile([P, 512], F32, tag="xsq")
                    nc.scalar.activation(r32(xsq[:]), x_g[:, j, :], AF.Square)
                    stat_sum(ps_ss[:], r32(xsq[:]), j == 0, j == DC - 1)
                m_row = stat.tile([1, 512], F32, tag="srowC")
                v_row = stat.tile([1, 512], F32, tag="srowD")
                t_row = stat.tile([1, 512], F32, tag="srowE")
                r_row = stat.tile([1, 512], F32, tag="srowF")
                nc.scalar.activation(m_row[:], ps_s[:], AF.Copy, scale=1.0 / D)
                nc.scalar.activation(v_row[:], ps_ss[:], AF.Copy, scale=1.0 / D)
                nc.vector.tensor_tensor(t_row[:], m_row[:], m_row[:], OP.mult)
                nc.vector.tensor_sub(v_row[:], v_row[:], t_row[:])
                nc.scalar.activation(v_row[:], v_row[:], AF.Sqrt, bias=LN_EPS)
                nc.vector.reciprocal(r_row[:], v_row[:])
                nc.vector.scalar_tensor_tensor(t_row[:], m_row[:], -1.0, r_row[:],
                                               OP.mult, OP.mult)
                rp = publish(r_row)
                mp = publish(t_row)
                ps_r = psAb.tile([P, 512], F32, tag="ps_r")
                ps_mrB = psAb.tile([P, 512], F32, tag="ps_mrB")
                bcast_row(ps_r[:], r32(rp[:]))
                bcast_row(ps_mrB[:], r32(mp[:]))
                xh_g = pA1.tile([P, DC, 512], F32, tag="xh_g")
                for j in range(DC):
                    nc.vector.tensor_tensor(xh_g[:, j, :], x_g[:, j, :], ps_r[:],
                                            OP.mult)
                    nc.vector.tensor_tensor(xh_g[:, j, :], xh_g[:, j, :],
                                            ps_mrB[:], OP.add)
                h_g = pA.tile([P, DC, 512], F32, tag="h_g")
                for j in range(DC):
                    ps_sj = psA.tile([P, 512], F32, tag="ps_sj")
                    ps_shj = psA.tile([P, 512], F32, tag="ps_shj")
                    for ki in range(DC):
                        nc.tensor.matmul(ps_sj[:],
                                         r32(wada1[:, ki, j * P:(j + 1) * P]),
                                         r32(sc_g[:, ki, :]),
                                         start=(ki == 0), stop=(ki == DC - 1))
                    for ki in range(DC):
                        nc.tensor.matmul(ps_shj[:],
                                         r32(wada1[:, ki, D + j * P:D + (j + 1) * P]),
                                         r32(sc_g[:, ki, :]),
                                         start=(ki == 0), stop=(ki == DC - 1))
                    tmp = pA.tile([P, 512], F32, tag="hfold")
                    nc.vector.scalar_tensor_tensor(
                        tmp[:], ps_sj[:], pk["one_p_bada_s"][:, j:j + 1],
                        xh_g[:, j, :], OP.add, OP.mult)
                    nc.vector.scalar_tensor_tensor(
                        r32(h_g[:, j, :]), ps_shj[:], pk["bada_sh"][:, j:j + 1],
                        tmp[:], OP.add, OP.add)
                nc.sync.dma_start(r32(htmp[:, :, gs]), r32(h_g[:]))

        # =========== Stage A2: qkv ===========
        if STAGE >= 2:
            ctx_kv = contextlib.ExitStack()
            pKV = ctx_kv.enter_context(tc.tile_pool(name="pKV", bufs=1))
            kT = pKV.tile([P, DC, L], F32, tag="kT")
            v65 = pKV.tile([P, KC, HEADS * 65], F32, tag="v65")
            qT = pKV.tile([P, DC, QT], F32, tag="qT")
            with tc.tile_pool(name="pA2", bufs=2) as pA2, \
                 tc.tile_pool(name="pA21", bufs=1) as pA21, \
                 tc.tile_pool(name="psA2", bufs=2, space="PSUM") as psA2, \
                 tc.tile_pool(name="psA2s", bufs=1, space="PSUM") as psA2s, \
                 tc.tile_pool(name="psA2b", bufs=1, space="PSUM") as psA2b:
                wv = pA21.tile([P, DC, D], F32, tag="wv")
                nc.sync.dma_start(r32(wv[:]), r32(Wqkv_c[:, :, 2 * D:3 * D]))
                for kc in range(KC):
                    vt = v65[:, kc, :]
                    dst = bass.AP(vt.tensor, vt.offset + DH,
                                  [list(vt.ap[0]), [65, HEADS]])
                    nc.vector.tensor_copy(r32(dst), ones[:, 0:HEADS])
                for g in range(NG):
                    gs = slice(g * 512, (g + 1) * 512)
                    h_g = pA21.tile([P, DC, 512], F32, tag="h_g2")
                    nc.sync.dma_start(r32(h_g[:]), r32(htmp[:, :, gs]))
                    # ---- k ----
                    ps_kss = psA2s.tile([1, 512], F32, tag="ps_kss")
                    for j in range(DC):
                        wk = pA2.tile([P, DC, P], F32, tag="wk")
                        nc.sync.dma_start(r32(wk[:]),
                                          r32(Wqkv_c[:, :, D + j * P:D + (j + 1) * P]))
                        ps_k = psA2.tile([P, 512], F32, tag="ps_k")
                        for ki in range(DC):
                            nc.tensor.matmul(ps_k[:], r32(wk[:, ki, :]),
                                             r32(h_g[:, ki, :]),
                                             start=(ki == 0), stop=(ki == DC - 1))
                        nc.vector.tensor_scalar_add(r32(kT[:, j, gs]), ps_k[:],
                                                    pk["bk"][:, j:j + 1])
                        ksq = pA2.tile([P, 512], F32, tag="ksq")
                        nc.scalar.activation(r32(ksq[:]), ps_k[:], AF.Square,
                                             bias=pk["bk"][:, j:j + 1])
                        stat_sum(ps_kss[:], r32(ksq[:]), j == 0, j == DC - 1)
                    rkp = recip_rows(ps_kss[:], 1e-12)
                    ps_rk = psA2b.tile([P, 512], F32, tag="ps_rk")
                    bcast_row(ps_rk[:], r32(rkp[:]))
                    for j in range(DC):
                        nc.vector.scalar_tensor_tensor(
                            r32(kT[:, j, gs]), kT[:, j, gs], pk["gk_s"][:, j:j + 1],
                            ps_rk[:], OP.mult, OP.mult)
                    # ---- v (token-major, 65 cols/head: [v_h | 1]) ----
                    for tt in range(g * 4, (g + 1) * 4):
                        lt = tt % 4
                        for n in range(2):
                            ps_v = psA2.tile([P, 384], F32, tag="ps_v")
                            for ki in range(DC):
                                nc.tensor.matmul(
                                    ps_v[:], r32(h_g[:, ki, lt * P:(lt + 1) * P]),
                                    r32(wv[:, ki, n * 384:(n + 1) * 384]),
                                    start=(ki == 0), stop=(ki == DC - 1))
                            vt = v65[:, tt, :]
                            dst = bass.AP(vt.tensor, vt.offset + n * 6 * 65,
                                          [list(vt.ap[0]), [65, 6], [1, DH]])
                            nc.vector.tensor_copy(
                                r32(dst), ps_v[:].rearrange("p (h d) -> p h d", h=6))
                    # ---- q (own tokens only) ----
                    if g == 0:
                        ps_qss = psA2s.tile([1, 512], F32, tag="ps_qss")
                        for j in range(DC):
                            wq = pA2.tile([P, DC, P], F32, tag="wq")
                            nc.sync.dma_start(r32(wq[:]),
                                              r32(Wqkv_c[:, :, j * P:(j + 1) * P]))
                            ps_q = psA2.tile([P, 512], F32, tag="ps_k")
                            for ki in range(DC):
                                nc.tensor.matmul(ps_q[:], r32(wq[:, ki, :]),
                                                 r32(h_g[:, ki, :]),
                                                 start=(ki == 0), stop=(ki == DC - 1))
                            nc.vector.tensor_scalar_add(r32(qT[:, j, :]), ps_q[:],
                                                        pk["bq"][:, j:j + 1])
                            qsq = pA2.tile([P, 512], F32, tag="ksq")
                            nc.scalar.activation(r32(qsq[:]), ps_q[:], AF.Square,
                                                 bias=pk["bq"][:, j:j + 1])
                            stat_sum(ps_qss[:], r32(qsq[:]), j == 0, j == DC - 1)
                        rqp = recip_rows(ps_qss[:], 1e-12)
                        ps_rq = psA2b.tile([P, 512], F32, tag="ps_rk")
                        bcast_row(ps_rq[:], r32(rqp[:]))
                        for j in range(DC):
                            nc.vector.scalar_tensor_tensor(
                                r32(qT[:, j, :]), qT[:, j, :], pk["gq_s"][:, j:j + 1],
                                ps_rq[:], OP.mult, OP.mult)

        if STAGE >= 3:
            # =========== Stage B: attention ===========
            attn = const.tile([P, DC, QT], F32, tag="attn")
            with tc.tile_pool(name="pB", bufs=3) as pB, \
                 tc.tile_pool(name="psB", bufs=3, space="PSUM") as psB, \
                 tc.tile_pool(name="psBa", bufs=2, space="PSUM") as psBa, \
                 tc.tile_pool(name="psBb", bufs=2, space="PSUM") as psBb:
                for h in range(HEADS):
                    po = (h % 2) * DH
                    ch = h // 2
                    ps_av = psBa.tile([65, 512], F32, tag="ps_av")
                    for kc in range(KC):
                        ks = slice(kc * P, (kc + 1) * P)
                        ps_sim = psB.tile([P, 512], F32, tag="ps_sim")
                        nc.tensor.matmul(ps_sim[:], r32(kT[po:po + DH, ch, ks]),
                                         r32(qT[po:po + DH, ch, :]),
                                         start=True, stop=True)
                        es = pB.tile([P, 512], F32, tag="es")
                        nc.scalar.activation(r32(es[:]), ps_sim[:], AF.Exp)
                        nc.tensor.matmul(ps_av[:],
                                         r32(v65[:, kc, h * 65:(h + 1) * 65]),
                                         r32(es[:]),
                                         start=(kc == 0), stop=(kc == KC - 1))
                    se_row = stat.tile([1, 512], F32, tag="srowA")
                    rec_row = stat.tile([1, 512], F32, tag="srowB")
                    nc.scalar.activation(se_row[:], ps_av[DH:DH + 1, :], AF.Copy)
                    nc.vector.reciprocal(rec_row[:], se_row[:])
                    rcp = publish(rec_row)
                    ps_rec = psBb.tile([DH, 512], F32, tag="ps_rec")
                    bcast_row(ps_rec[:], r32(rcp[:]), m=DH)
                    rec_b = pB.tile([DH, 512], F32, tag="rec_b")
                    nc.vector.tensor_copy(rec_b[:], ps_rec[:])
                    nc.vector.tensor_tensor(r32(attn[po:po + DH, ch, :]),
                                            ps_av[0:DH, :], rec_b[:], OP.mult)
        if STAGE >= 2:
            ctx_kv.close()   # free kT / v65 / qT

        if STAGE >= 4:
            # =========== Stage C: to_out + rmsnorm_o + residual ===========
            late = ctx.enter_context(tc.tile_pool(name="late", bufs=1))
            x2 = late.tile([P, DC, QT], F32, tag="x2")
            x_own = late.tile([P, DC, QT], F32, tag="x_own")
            h2 = late.tile([P, DC, QT], F32, tag="h2")
            nc.sync.dma_start(x_own[:], xTo_c)
            with tc.tile_pool(name="pC", bufs=2) as pC, \
                 tc.tile_pool(name="pC1", bufs=1) as pC1, \
                 tc.tile_pool(name="psC", bufs=2, space="PSUM") as psC, \
                 tc.tile_pool(name="psCs", bufs=1, space="PSUM") as psCs, \
                 tc.tile_pool(name="psCb", bufs=1, space="PSUM") as psCb:
                ps_yss = psCs.tile([1, 512], F32, tag="ps_yss")
                y_sb = pC1.tile([P, DC, 512], F32, tag="y_sb")
                for j in range(DC):
                    wo = pC.tile([P, DC, P], F32, tag="wo")
                    nc.sync.dma_start(r32(wo[:]), r32(Wo_c[:, :, j * P:(j + 1) * P]))
                    ps_y = psC.tile([P, 512], F32, tag="ps_y")
                    for ki in range(DC):
                        nc.tensor.matmul(ps_y[:], r32(wo[:, ki, :]), r32(attn[:, ki, :]),
                                         start=(ki == 0), stop=(ki == DC - 1))
                    nc.vector.tensor_scalar(y_sb[:, j, :], ps_y[:],
                                            pk["bo_eff"][:, j:j + 1],
                                            pk["go_s"][:, j:j + 1], OP.add, OP.mult)
                    ysq = pC.tile([P, 512], F32, tag="ysq")
                    nc.scalar.activation(r32(ysq[:]), ps_y[:], AF.Square,
                                         bias=pk["bo_eff"][:, j:j + 1])
                    stat_sum(ps_yss[:], r32(ysq[:]), j == 0, j == DC - 1)
                rop = recip_rows(ps_yss[:], 1e-12)
                ps_ro = psCb.tile([P, 512], F32, tag="ps_ro")
                bcast_row(ps_ro[:], r32(rop[:]))
                for j in range(DC):
                    wga = pC.tile([P, DC, P], F32, tag="wo")
                    nc.sync.dma_start(r32(wga[:]),
                                      r32(Wada_c[:, :, 2 * D + j * P:2 * D + (j + 1) * P]))
                    ps_ga = psC.tile([P, 512], F32, tag="ps_y")
                    for ki in range(DC):
                        nc.tensor.matmul(ps_ga[:], r32(wga[:, ki, :]),
                                         r32(sc_own[:, ki, :]),
                                         start=(ki == 0), stop=(ki == DC - 1))
                    t = pC.tile([P, 512], F32, tag="cfold")
                    nc.vector.tensor_tensor(t[:], y_sb[:, j, :], ps_ro[:], OP.mult)
                    nc.vector.scalar_tensor_tensor(t[:], ps_ga[:],
                                                   pk["bada_ga"][:, j:j + 1], t[:],
                                                   OP.add, OP.mult)
                    nc.vector.tensor_tensor(x2[:, j, :], x_own[:, j, :], t[:], OP.add)

        if STAGE >= 5:
            # =========== Stage D: LN2 + mod2(s/sh) + h2 ===========
            with tc.tile_pool(name="pD", bufs=2) as pD, \
                 tc.tile_pool(name="pD1", bufs=1) as pD1, \
                 tc.tile_pool(name="psD", bufs=2, space="PSUM") as psD, \
                 tc.tile_pool(name="psDs", bufs=1, space="PSUM") as psDs, \
                 tc.tile_pool(name="psDb", bufs=1, space="PSUM") as psDb:
                ps_s2 = psDs.tile([1, 512], F32, tag="ps_s2")
                ps_ss2 = psDs.tile([1, 512], F32, tag="ps_ss2")
                x2r = pD1.tile([P, DC, 512], F32, tag="x2r")
                nc.vector.tensor_copy(r32(x2r[:]), x2[:])
                for j in range(DC):
                    stat_sum(ps_s2[:], r32(x2r[:, j, :]), j == 0, j == DC - 1)
                for j in range(DC):
                    x2sq = pD.tile([P, 512], F32, tag="x2sq")
                    nc.scalar.activation(r32(x2sq[:]), x2[:, j, :], AF.Square)
                    stat_sum(ps_ss2[:], r32(x2sq[:]), j == 0, j == DC - 1)
                m_row = stat.tile([1, 512], F32, tag="srowC")
                v_row = stat.tile([1, 512], F32, tag="srowD")
                t_row = stat.tile([1, 512], F32, tag="srowE")
                r_row = stat.tile([1, 512], F32, tag="srowF")
                nc.scalar.activation(m_row[:], ps_s2[:], AF.Copy, scale=1.0 / D)
                nc.scalar.activation(v_row[:], ps_ss2[:], AF.Copy, scale=1.0 / D)
                nc.vector.tensor_tensor(t_row[:], m_row[:], m_row[:], OP.mult)
                nc.vector.tensor_sub(v_row[:], v_row[:], t_row[:])
                nc.scalar.activation(v_row[:], v_row[:], AF.Sqrt, bias=LN_EPS)
                nc.vector.reciprocal(r_row[:], v_row[:])
                nc.vector.scalar_tensor_tensor(t_row[:], m_row[:], -1.0, r_row[:],
                                               OP.mult, OP.mult)
                rp = publish(r_row)
                mp = publish(t_row)
                ps_r2 = psDb.tile([P, 512], F32, tag="ps_r2")
                ps_mrB2 = psDb.tile([P, 512], F32, tag="ps_mrB2")
                bcast_row(ps_r2[:], r32(rp[:]))
                bcast_row(ps_mrB2[:], r32(mp[:]))
                x2h = pD1.tile([P, DC, 512], F32, tag="x2h")
                for j in range(DC):
                    nc.vector.tensor_tensor(x2h[:, j, :], x2[:, j, :], ps_r2[:], OP.mult)
                    nc.vector.tensor_tensor(x2h[:, j, :], x2h[:, j, :], ps_mrB2[:],
                                            OP.add)
                for j in range(DC):
                    wsm = pD.tile([P, DC, P], F32, tag="wsm")
                    wshm = pD.tile([P, DC, P], F32, tag="wshm")
                    nc.sync.dma_start(r32(wsm[:]),
                                      r32(Wada_c[:, :, 3 * D + j * P:3 * D + (j + 1) * P]))
                    nc.sync.dma_start(r32(wshm[:]),
                                      r32(Wada_c[:, :, 4 * D + j * P:4 * D + (j + 1) * P]))
                    ps_sm = psD.tile([P, 512], F32, tag="ps_sm")
                    ps_shm = psD.tile([P, 512], F32, tag="ps_shm")
                    for ki in range(DC):
                        nc.tensor.matmul(ps_sm[:], r32(wsm[:, ki, :]),
                                         r32(sc_own[:, ki, :]),
                                         start=(ki == 0), stop=(ki == DC - 1))
                    for ki in range(DC):
                        nc.tensor.matmul(ps_shm[:], r32(wshm[:, ki, :]),
                                         r32(sc_own[:, ki, :]),
                                         start=(ki == 0), stop=(ki == DC - 1))
                    tmp = pD.tile([P, 512], F32, tag="h2fold")
                    nc.vector.scalar_tensor_tensor(
                        tmp[:], ps_sm[:], pk["one_p_bada_sm"][:, j:j + 1],
                        x2h[:, j, :], OP.add, OP.mult)
                    nc.vector.scalar_tensor_tensor(
                        r32(h2[:, j, :]), ps_shm[:], pk["bada_shm"][:, j:j + 1],
                        tmp[:], OP.add, OP.add)

        if STAGE >= 6:
            # =========== Stage E: MLP + g_mlp + output ===========
            with tc.tile_pool(name="pE", bufs=2) as pE, \
                 tc.tile_pool(name="pE1", bufs=1) as pE1, \
                 tc.tile_pool(name="psE", bufs=2, space="PSUM") as psE:
                ms = pE1.tile([P, FFC, 512], F32, tag="ms")
                for f in range(FFC):
                    w1 = pE.tile([P, DC, P], F32, tag="w1")
                    nc.sync.dma_start(r32(w1[:]), r32(W1_c[:, :, f * P:(f + 1) * P]))
                    ps_m1 = psE.tile([P, 512], F32, tag="ps_m1")
                    for ki in range(DC):
                        nc.tensor.matmul(ps_m1[:], r32(w1[:, ki, :]), r32(h2[:, ki, :]),
                                         start=(ki == 0), stop=(ki == DC - 1))
                    if SIM_COMPAT:
                        nc.scalar.activation(r32(ms[:, f, :]), ps_m1[:], AF.Sigmoid,
                                             bias=b1_pp[:, f:f + 1])
                        nc.vector.scalar_tensor_tensor(
                            r32(ms[:, f, :]), ps_m1[:], b1_pp[:, f:f + 1],
                            ms[:, f, :], OP.add, OP.mult)
                    else:
                        nc.scalar.activation(r32(ms[:, f, :]), ps_m1[:], AF.Silu,
                                             bias=b1_pp[:, f:f + 1])
                gm_sb = pE1.tile([P, DC, 512], F32, tag="gm_sb")
                out_sb = pE1.tile([P, DC, 512], F32, tag="out_sb")
                for j in range(DC):
                    wgm = pE.tile([P, DC, P], F32, tag="wgm")
                    nc.sync.dma_start(r32(wgm[:]),
                                      r32(Wada_c[:, :, 5 * D + j * P:5 * D + (j + 1) * P]))
                    ps_gm = psE.tile([P, 512], F32, tag="ps_gm")
                    for ki in range(DC):
                        nc.tensor.matmul(ps_gm[:], r32(wgm[:, ki, :]),
                                         r32(sc_own[:, ki, :]),
                                         start=(ki == 0), stop=(ki == DC - 1))
                    nc.vector.tensor_scalar_add(gm_sb[:, j, :], ps_gm[:],
                                                pk["bada_gm"][:, j:j + 1])
                for j in range(DC):
                    w2 = pE.tile([P, FFC, P], F32, tag="w2")
                    nc.sync.dma_start(r32(w2[:]), r32(W2_c[:, :, j * P:(j + 1) * P]))
                    ps_o = psE.tile([P, 512], F32, tag="ps_m1")
                    for f in range(FFC):
                        nc.tensor.matmul(ps_o[:], r32(w2[:, f, :]), r32(ms[:, f, :]),
                                         start=(f == 0), stop=(f == FFC - 1))
                    t = pE.tile([P, 512], F32, tag="efold")
                    nc.vector.scalar_tensor_tensor(t[:], ps_o[:], pk["b2"][:, j:j + 1],
                                                   gm_sb[:, j, :], OP.add, OP.mult)
                    nc.vector.tensor_tensor(out_sb[:, j, :], x2[:, j, :], t[:], OP.add)
                nc.sync.dma_start(outT_c, out_sb[:])
        if STAGE < 6:
            nc.sync.dma_start(outT_c, sc_own[:])


_BUILT = None


def _register_const(nc, value):
    t = nc.alloc_sbuf_tensor(f"const-float32-{value}", [128, 1], F32)
    nc.gpsimd.memset(t.ap(), value)
    nc.const_aps.aps[(F32, value)] = t.ap()
    nc.all_engine_barrier()


def _build():
    global _BUILT
    if _BUILT is None:
        nc = bacc.Bacc("TRN2", target_bir_lowering=False, debug=False,
                       num_devices=N_CORES)
        _register_const(nc, LN_EPS)
        _emit(nc)
        nc.compile()
        _BUILT = nc
    return _BUILT


def _pack_pp(vec, nchunk):
    return np.ascontiguousarray(np.asarray(vec, np.float32).reshape(nchunk, P).T)


def make_in_maps(inputs):
    f = lambda k: np.asarray(inputs[k], dtype=np.float32)
    x, c = f("x"), f("c")
    b_ada, b_qkv, b_o = f("b_ada"), f("b_qkv"), f("b_o")
    bo_eff = b_o + b_qkv[2 * D:] @ f("W_o")   # v-bias folded through attention
    shared = {
        "Wada": f("W_ada"), "Wqkv": f("W_qkv"), "Wo": f("W_o"),
        "W1": f("W1"), "W2": f("W2"),
        "ONES": np.ones((P, 512), np.float32),
        "one_p_bada_s": _pack_pp(1.0 + b_ada[:D], DC),
        "bada_sh": _pack_pp(b_ada[D:2 * D], DC),
        "bada_ga": _pack_pp(b_ada[2 * D:3 * D], DC),
        "one_p_bada_sm": _pack_pp(1.0 + b_ada[3 * D:4 * D], DC),
        "bada_shm": _pack_pp(b_ada[4 * D:5 * D], DC),
        "bada_gm": _pack_pp(b_ada[5 * D:], DC),
        "bk": _pack_pp(b_qkv[D:2 * D], DC),
        "bq": _pack_pp(b_qkv[:D], DC),
        "bo_eff": _pack_pp(bo_eff, DC),
        "go_s": _pack_pp(f("g_o") * SQRT_D, DC),
        "gq_s": _pack_pp(f("g_q") * SQRT_D * SCALE, DC),
        "gk_s": _pack_pp(f("g_k") * SQRT_D, DC),
        "b2": _pack_pp(f("b2"), DC),
        "b1": _pack_pp(f("b1"), FFC),
    }
    in_maps = []
    for core in range(N_CORES):
        b = core // 4
        q0 = (core % 4) * QT
        perm = np.r_[q0:q0 + QT, 0:q0, q0 + QT:L]
        m = dict(shared)
        m["xT"] = np.ascontiguousarray(x[b][perm].T)
        m["cT"] = np.ascontiguousarray(c[b][perm].T)
        m["xT_own"] = np.ascontiguousarray(x[b][q0:q0 + QT].T)
        in_maps.append(m)
    return in_maps


def assemble_out(results):
    out = np.empty((B, L, D), dtype=np.float32)
    for core in range(N_CORES):
        b = core // 4
        q0 = (core % 4) * QT
        out[b, q0:q0 + QT] = results[core]["outT"].T
    return out


def kernel(**inputs):
    nc = _build()
    in_maps = make_in_maps(inputs)
    res = run_bass_kernel_spmd(nc, in_maps, core_ids=list(range(N_CORES)))
    return assemble_out(res.results)

